# revision 1
# baseline (speedup 1.0000x reference)
"""Trainium2 Bass kernel for nn_LogisticRegression (embedding_lookup).

Reference computation (B=1024, S=200, V=50000, E=300):
    x1 = one-hot presence over vocab (duplicates set once)      [B, V]
    emb_mean = mean(emb_table[x], axis=1)                       [B, E]
    logits = concat([emb_mean, x1]) @ W.T + b                   [B, 1]
    out = sigmoid(logits)

Algebraic restructure (never materializes x1 / feats):
    t[v]     = emb_table[v] . W[0, :E] / S
    logit[i] = sum_j t[x[i,j]] + sum_j m[i,j] * W_voc[x[i,j]] + b
where m is the first-occurrence mask (dedup = the one-hot .set semantics).

Device plan (single NEFF, SPMD on 8 cores):
  phase 1 (vocab-sharded): core c computes t over its 6250-row table slice
           (reads 7.5MB of the 60MB table), pairs it with its W_voc slice.
  AllGather: 50KB/core (t, w) pair table -> full 401KB table on every core.
  phase 2 (batch-sharded): core c handles 128 batch rows. The (t, w) pair
           of token (p, j) is fetched with dma_gather at 256-byte block
           granularity (block = pair_idx // 32, fits int16; token slot
           s = j*128 + p lands on partition p = its batch row). Each
           gathered [128, 8, 64] slab is folded into per-row logits by a
           fused multiply-accumulate against a host-built one-hot weight
           slab (bf16, exact 0/1/m values):
              wv[p, j, 2*(pair%32)]   = 1      (selects t)
              wv[p, j, 2*(pair%32)+1] = m[p,j] (selects w, pre-masked)
  Finally sigmoid(logit + b) per row.

Empirical ground rules for this stack (established by direct HW tests):
  * indirect_dma_start runs ~10ns/descriptor serialized -> unusable here;
  * dma_gather works exactly, but <= 1024 indices per instruction;
  * tensor_tensor_reduce / tensor_scalar(accum_out) crash the compiled
    NEFF; scalar_tensor_tensor(accum_out) is exercised by USE_STT below.

Host side only shards tensors and precomputes integer index data (gather
block ids, one-hot selection weights, first-occurrence mask) from the int
token ids, then concatenates the per-core outputs.
"""

import sys

if "/opt/trn_rl_repo" not in sys.path:
    sys.path.insert(0, "/opt/trn_rl_repo")

# This image's antenv package lacks the optional axon_hooks module, but
# concourse.bass_utils imports it unconditionally on the BASS_TRACE path.
# Provide a compatible stub so tracing degrades gracefully instead of
# crashing; a harness may install a real hook via set_axon_ntff_profile_hook.
try:
    import antenv.axon_hooks  # noqa: F401
except ImportError:
    import types as _types

    import antenv as _antenv

    _hooks_mod = _types.ModuleType("antenv.axon_hooks")
    _hooks_mod._hook = None

    def _set_hook(h, _m=_hooks_mod):
        _m._hook = h

    def _get_hook(_m=_hooks_mod):
        return _m._hook

    _hooks_mod.set_axon_ntff_profile_hook = _set_hook
    _hooks_mod.get_axon_ntff_profile_hook = _get_hook
    sys.modules["antenv.axon_hooks"] = _hooks_mod
    _antenv.axon_hooks = _hooks_mod

import ml_dtypes
import numpy as np

from concourse import bacc, bass, mybir, tile
from concourse.bass_utils import run_bass_kernel_spmd

# Problem shapes (hardcoded per contract).
N_CORES = 8
B = 1024
S = 200
V = 50000
E = 300

RPC = B // N_CORES          # batch rows per core = 128
VPC = V // N_CORES          # vocab rows per core = 6250
KC = 49                     # free-dim columns of the per-core t layout
VPAD = KC * 128             # padded vocab rows per core = 6272
NPAIR = N_CORES * VPAD      # total (t, w) pairs after AllGather = 50176
TCHUNK = 7                  # table tiles per phase-1 DMA chunk
NCHUNK = KC // TCHUNK       # 7 chunks of 7 tiles

# phase-2 gather geometry
BPAIR = 32                  # pairs per 256B gather block
NBLK = NPAIR // BPAIR       # 1568 blocks
ESZ = 2 * BPAIR             # 64 f32 per block
GI = 1024                   # indices per dma_gather (HW limit)
NG = RPC * S // GI          # 25 gather instructions
JPG = GI // RPC             # 8 j-columns per gather

USE_STT = True              # fused (g*wv -> accum) on DVE

_BUILT = None
LAST_RUN = None  # BassKernelResults of the most recent launch (for harness)


def _build():
    f32 = mybir.dt.float32
    bf16 = mybir.dt.bfloat16
    i16 = mybir.dt.int16
    nc = bacc.Bacc("TRN2", target_bir_lowering=False, debug=False,
                   num_devices=N_CORES)

    tbl = nc.dram_tensor("tbl", [VPAD, E], f32, kind="ExternalInput")
    wemb = nc.dram_tensor("wemb", [1, E], f32, kind="ExternalInput")
    wvoc = nc.dram_tensor("wvoc", [128, KC], f32, kind="ExternalInput")
    gidx = nc.dram_tensor("gidx", [128, RPC * S // 16], i16, kind="ExternalInput")
    wv = nc.dram_tensor("wv", [RPC, S, ESZ], bf16, kind="ExternalInput")
    bias = nc.dram_tensor("bias", [1, 1], f32, kind="ExternalInput")
    outp = nc.dram_tensor("outp", [RPC, 1], f32, kind="ExternalOutput")

    with tile.TileContext(nc) as tc:
        with tc.tile_pool(name="dram", bufs=1, space="DRAM") as dram, \
             tc.tile_pool(name="sbuf", bufs=1) as sb1, \
             tc.tile_pool(name="ld", bufs=3) as ld, \
             tc.tile_pool(name="gbl", bufs=4) as gbl, \
             tc.tile_pool(name="scr", bufs=2) as scr:
            u_slice = dram.tile([VPAD, 2], f32)
            u_full = dram.tile([NBLK, ESZ], f32)

            # --- small input loads (overlap the table read) ---
            wemb_sb = sb1.tile([128, E], f32)
            nc.scalar.dma_start(wemb_sb[:], wemb.ap().partition_broadcast(128))
            # fold the 1/S of the sequence mean into the embedding weights
            nc.vector.tensor_scalar_mul(wemb_sb[:], wemb_sb[:], 1.0 / S)
            wvoc_sb = sb1.tile([128, KC], f32)
            nc.scalar.dma_start(wvoc_sb[:], wvoc.ap())
            gidx_sb = sb1.tile([128, RPC * S // 16], i16)
            nc.scalar.dma_start(gidx_sb[:], gidx.ap())
            wv_sb = sb1.tile([RPC, S, ESZ], bf16)
            nc.scalar.dma_start(wv_sb[:], wv.ap())
            b_sb = sb1.tile([128, 1], f32)
            nc.scalar.dma_start(b_sb[:], bias.ap().partition_broadcast(128))

            # u_sb[p, k, 0] = t[slice row 128k+p]/S ; u_sb[p, k, 1] = W_voc
            u_sb = sb1.tile([128, KC, 2], f32)

            # --- phase 1: t = tbl @ wemb / S, one mult + reduce per chunk ---
            wemb_bc = wemb_sb[:].unsqueeze(1).to_broadcast([128, TCHUNK, E])
            for ch in range(NCHUNK):
                rows = TCHUNK * 128
                chunk = ld.tile([128, TCHUNK, E], f32, tag="tblchunk")
                src = tbl.ap()[ch * rows:(ch + 1) * rows, :]
                nc.sync.dma_start(chunk[:], src.rearrange("(t p) e -> p t e", p=128))
                prod = scr.tile([128, TCHUNK, E], f32, tag="prod")
                nc.vector.tensor_tensor(
                    out=prod[:], in0=chunk[:], in1=wemb_bc,
                    op=mybir.AluOpType.mult)
                nc.vector.tensor_reduce(
                    out=u_sb[:, ch * TCHUNK:(ch + 1) * TCHUNK, 0],
                    in_=prod[:], axis=mybir.AxisListType.X,
                    op=mybir.AluOpType.add)
            nc.vector.tensor_copy(out=u_sb[:, :, 1], in_=wvoc_sb[:])
            nc.gpsimd.dma_start(u_slice[:], u_sb[:])

            # --- all-gather the (t, w) pair table ---
            nc.gpsimd.collective_compute(
                "AllGather",
                mybir.AluOpType.bypass,
                replica_groups=[list(range(N_CORES))],
                ins=[u_slice.opt()],
                outs=[u_full.opt()],
            )

            # --- phase 2: block-gather + fused extract/reduce per slab ---
            acc = sb1.tile([128, NG], f32)
            for k in range(NG):
                g = gbl.tile([128, JPG, ESZ], f32, tag="gblk")
                nc.gpsimd.dma_gather(
                    g[:], u_full[:],
                    gidx_sb[:, (GI // 16) * k:(GI // 16) * (k + 1)],
                    num_idxs=GI, num_idxs_reg=GI, elem_size=ESZ,
                )
                wv_k = wv_sb[:, JPG * k:JPG * (k + 1), :]
                if USE_STT:
                    po = scr.tile([128, JPG, ESZ], f32, tag="po")
                    nc.vector.scalar_tensor_tensor(
                        out=po[:], in0=g[:], scalar=1.0, in1=wv_k,
                        op0=mybir.AluOpType.mult, op1=mybir.AluOpType.mult,
                        accum_out=acc[:, k:k + 1])
                else:
                    po = scr.tile([128, JPG, ESZ], f32, tag="po")
                    nc.vector.tensor_tensor(
                        out=po[:], in0=g[:], in1=wv_k,
                        op=mybir.AluOpType.mult)
                    nc.vector.tensor_reduce(
                        out=acc[:, k:k + 1], in_=po[:],
                        axis=mybir.AxisListType.XY, op=mybir.AluOpType.add)

            logit = sb1.tile([128, 1], f32)
            nc.vector.tensor_reduce(
                out=logit[:], in_=acc[:], axis=mybir.AxisListType.X,
                op=mybir.AluOpType.add)
            res = sb1.tile([128, 1], f32)
            nc.scalar.activation(
                out=res[:], in_=logit[:],
                func=mybir.ActivationFunctionType.Sigmoid,
                bias=b_sb[:], scale=1.0)
            nc.scalar.dma_start(outp.ap(), res[:])

    nc.compile()
    return nc


def _first_occurrence_mask(xr: np.ndarray) -> np.ndarray:
    """m[i, j] = 1 iff x[i, j] does not appear at any k < j in row i."""
    eq = xr[:, :, None] == xr[:, None, :]          # [rows, S, S]
    dup = np.tril(eq, -1).any(axis=2)              # seen earlier in the row
    return ~dup


def kernel(x, emb_table, W, b):
    global _BUILT, LAST_RUN
    if _BUILT is None:
        _BUILT = _build()
    nc = _BUILT

    x = np.asarray(x)
    emb_table = np.ascontiguousarray(np.asarray(emb_table, dtype=np.float32))
    W = np.asarray(W, dtype=np.float32)
    b = np.asarray(b, dtype=np.float32)

    wemb = np.ascontiguousarray(W[:, :E])                  # [1, E]
    wv_full = W[0, E:]                                     # [V]
    bias_np = b.reshape(1, 1)

    in_maps = []
    for c in range(N_CORES):
        tbl = np.zeros((VPAD, E), dtype=np.float32)
        tbl[:VPC] = emb_table[c * VPC:(c + 1) * VPC]
        wvs = np.zeros(VPAD, dtype=np.float32)
        wvs[:VPC] = wv_full[c * VPC:(c + 1) * VPC]
        wvoc_sh = np.ascontiguousarray(wvs.reshape(KC, 128).T)  # [128, KC]

        xr = x[c * RPC:(c + 1) * RPC].astype(np.int64)          # [RPC, S]
        ct = xr // VPC
        r = xr - ct * VPC
        # global pair index (matches the phase-1 SBUF->DRAM flat layout)
        pidx = ct * VPAD + (r % 128) * KC + (r // 128)          # [RPC, S]
        m = _first_occurrence_mask(xr)                          # [RPC, S] bool

        # gather block ids, wrapped in 16 partitions, replicated x8
        blk = (pidx // BPAIR).astype(np.int16)                  # [RPC, S]
        s = np.arange(RPC * S)
        w16 = np.zeros((16, RPC * S // 16), dtype=np.int16)
        w16[s % 16, s // 16] = blk[s % RPC, s // RPC]           # slot s=(j*128+p)
        gidx_np = np.tile(w16, (8, 1))                          # [128, 1600]

        # one-hot extraction weights (exact in bf16)
        woff = (pidx % BPAIR) * 2                               # [RPC, S]
        wv_np = np.zeros((RPC, S, ESZ), dtype=ml_dtypes.bfloat16)
        rows = np.arange(RPC)[:, None]
        cols = np.arange(S)[None, :]
        wv_np[rows, cols, woff] = 1.0
        wv_np[rows, cols, woff + 1] = m.astype(ml_dtypes.bfloat16)

        in_maps.append({
            "tbl": tbl,
            "wemb": wemb,
            "wvoc": wvoc_sh,
            "gidx": gidx_np,
            "wv": wv_np,
            "bias": bias_np,
        })

    LAST_RUN = run_bass_kernel_spmd(nc, in_maps, core_ids=list(range(N_CORES)))
    out = np.concatenate(
        [LAST_RUN.results[c]["outp"].reshape(RPC) for c in range(N_CORES)]
    )
    return out.reshape(B, 1)



# revision 2
# speedup vs baseline: 2.6442x; 2.6442x over previous
"""Trainium2 Bass kernel for nn_LogisticRegression (embedding_lookup).

Reference computation (B=1024, S=200, V=50000, E=300):
    x1 = one-hot presence over vocab (duplicates set once)      [B, V]
    emb_mean = mean(emb_table[x], axis=1)                       [B, E]
    logits = concat([emb_mean, x1]) @ W.T + b                   [B, 1]
    out = sigmoid(logits)

Algebraic restructure (never materializes x1 / feats):
    t[v]     = emb_table[v] . W[0, :E] / S
    w[v]     = W[0, E + v]
    logit[i] = sum_v cnt[i,v]*t[v] + sum_v pres[i,v]*w[v] + b
             = sum_v pres[i,v]*(t[v]+w[v]) + sum_dups (cnt-pres)*t[v] + b
where pres is the 0/1 presence matrix and the dup correction covers the
rare tokens repeated within a row (~0.4 per row).

Device plan (single NEFF, SPMD on 8 cores, vocab-sharded):
  Core c owns vocab rows [c*6250, (c+1)*6250) padded to 6272 = 49*128.
  phase 1: stream the 7.5MB table slice; per column k a fused DVE
           scalar_tensor_tensor computes t[:, k] = sum_e tbl*We/S with
           accum_out; u = t + wvoc.  (u[p, k] = t+w of vocab row 128k+p)
  phase 2: logits as a DENSE matmul against a host-built fp8 presence
           matrix (exact 0/1 values): for each k,
              psum[1, 1024] += u_bf[:, k].T @ presT[k-chunk][128, 1024]
           49 chunks accumulate in 2 PSUM banks.  This replaces the
           per-token SWDGE dma_gather of the previous version (which
           serialized ~8.4ns/index on the GpSimd Q7 path = 216us).
  dup fix: one 128-slot dma_gather pulls 256B blocks of the t table,
           a DVE one-hot extract forms val[s] = d_s * t[p_s], and a
           [128,1].T @ [128,1024] matmul spreads the corrections onto
           the same PSUM accumulators.
  finish:  ReduceScatter(add) of the [1024] partial logits (4KB); core c
           receives rows [128c, 128c+128), applies sigmoid(x + b), and
           writes its 128 outputs.  Host concatenation is a plain
           reorder of integer-indexed slices.

Empirical ground rules for this stack (established by direct HW tests):
  * indirect_dma_start ~10ns/descriptor serialized; dma_gather ~8.4ns/idx
    on the GpSimd software-DGE path -> avoid bulk gathers entirely;
  * tensor_tensor_reduce / tensor_scalar(accum_out) crash the compiled
    NEFF; scalar_tensor_tensor(accum_out) works and is used here.

Host side only shards tensors and precomputes integer index data (the
presence bitmap, dup slots, gather block ids) from the int token ids,
then concatenates the per-core outputs.
"""

import sys

if "/opt/trn_rl_repo" not in sys.path:
    sys.path.insert(0, "/opt/trn_rl_repo")

# This image's antenv package lacks the optional axon_hooks module, but
# concourse.bass_utils imports it unconditionally on the BASS_TRACE path.
# Provide a compatible stub so tracing degrades gracefully instead of
# crashing; a harness may install a real hook via set_axon_ntff_profile_hook.
try:
    import antenv.axon_hooks  # noqa: F401
except ImportError:
    import types as _types

    import antenv as _antenv

    _hooks_mod = _types.ModuleType("antenv.axon_hooks")
    _hooks_mod._hook = None

    def _set_hook(h, _m=_hooks_mod):
        _m._hook = h

    def _get_hook(_m=_hooks_mod):
        return _m._hook

    _hooks_mod.set_axon_ntff_profile_hook = _set_hook
    _hooks_mod.get_axon_ntff_profile_hook = _get_hook
    sys.modules["antenv.axon_hooks"] = _hooks_mod
    _antenv.axon_hooks = _hooks_mod

import ml_dtypes
import numpy as np

from concourse import bacc, bass, mybir, tile
from concourse.bass_utils import run_bass_kernel_spmd

# Problem shapes (hardcoded per contract).
N_CORES = 8
B = 1024
S = 200
V = 50000
E = 300

VPC = V // N_CORES          # vocab rows per core = 6250
KC = 49                     # u columns; padded vocab rows = 128*KC = 6272
VPAD = KC * 128
TCH = 7                     # k-columns per streamed chunk group
NCH = KC // TCH             # 7 chunk groups
DUPN = 128                  # dup-correction slots per core
GBLK = 64                   # f32 per 256B gather block

_BUILT = None
LAST_RUN = None  # BassKernelResults of the most recent launch (for harness)


def _build():
    f32 = mybir.dt.float32
    bf16 = mybir.dt.bfloat16
    fp8 = mybir.dt.float8e4
    i16 = mybir.dt.int16
    nc = bacc.Bacc("TRN2", target_bir_lowering=False, debug=False,
                   num_devices=N_CORES)

    tbl = nc.dram_tensor("tbl", [VPAD, E], f32, kind="ExternalInput")
    wemb = nc.dram_tensor("wemb", [1, E], f32, kind="ExternalInput")
    wvoc = nc.dram_tensor("wvoc", [128, KC], f32, kind="ExternalInput")
    pres = nc.dram_tensor("pres", [KC, 128, B], fp8, kind="ExternalInput")
    didx = nc.dram_tensor("didx", [128, DUPN // 16], i16, kind="ExternalInput")
    dwv = nc.dram_tensor("dwv", [128, 1, GBLK], bf16, kind="ExternalInput")
    dspread = nc.dram_tensor("dspread", [128, B], bf16, kind="ExternalInput")
    bias = nc.dram_tensor("bias", [1, 1], f32, kind="ExternalInput")
    outp = nc.dram_tensor("outp", [1, 128], f32, kind="ExternalOutput")

    with tile.TileContext(nc) as tc:
        with tc.tile_pool(name="dram", bufs=1, space="DRAM") as dram, \
             tc.tile_pool(name="sbuf", bufs=1) as sb1, \
             tc.tile_pool(name="tb", bufs=3) as tb, \
             tc.tile_pool(name="pr", bufs=3) as pr, \
             tc.tile_pool(name="scr", bufs=2) as scr, \
             tc.tile_pool(name="ps", bufs=1, space="PSUM") as ps:
            t_dram = dram.tile([VPAD // GBLK, GBLK], f32)
            accd = dram.tile([1, B], f32)
            rsd = dram.tile([1, B // N_CORES], f32)

            # --- small input loads (overlap the table read) ---
            wemb_sb = sb1.tile([128, E], f32)
            nc.gpsimd.dma_start(wemb_sb[:], wemb.ap().partition_broadcast(128))
            # fold the 1/S of the sequence mean into the embedding weights
            nc.vector.tensor_scalar_mul(wemb_sb[:], wemb_sb[:], 1.0 / S)
            wvoc_sb = sb1.tile([128, KC], f32)
            nc.gpsimd.dma_start(wvoc_sb[:], wvoc.ap())
            didx_sb = sb1.tile([128, DUPN // 16], i16)
            nc.gpsimd.dma_start(didx_sb[:], didx.ap())
            dwv_sb = sb1.tile([128, 1, GBLK], bf16)
            nc.gpsimd.dma_start(dwv_sb[:], dwv.ap())
            dspread_sb = sb1.tile([128, B], bf16)
            nc.gpsimd.dma_start(dspread_sb[:], dspread.ap())
            b_sb = sb1.tile([1, 1], f32)
            nc.gpsimd.dma_start(b_sb[:], bias.ap())

            t_raw = sb1.tile([128, KC], f32)
            u_bf = sb1.tile([128, KC], bf16)
            psA = ps.tile([1, B // 2], f32)
            psB = ps.tile([1, B // 2], f32)

            # --- phases 1+2 interleaved per chunk group of 7 k-columns ---
            for ch in range(NCH):
                rows = TCH * 128
                chunk = tb.tile([128, TCH, E], f32, tag="tblchunk")
                src = tbl.ap()[ch * rows:(ch + 1) * rows, :]
                nc.sync.dma_start(chunk[:], src.rearrange("(t p) e -> p t e", p=128))
                pchunk = pr.tile([128, TCH, B], fp8, tag="preschunk")
                nc.scalar.dma_start(
                    pchunk[:],
                    pres.ap()[ch * TCH:(ch + 1) * TCH].rearrange("k p b -> p k b"))
                for t in range(TCH):
                    k = ch * TCH + t
                    po = scr.tile([128, E], f32, tag="po")
                    nc.vector.scalar_tensor_tensor(
                        out=po[:], in0=chunk[:, t, :], scalar=1.0,
                        in1=wemb_sb[:],
                        op0=mybir.AluOpType.mult, op1=mybir.AluOpType.mult,
                        accum_out=t_raw[:, k:k + 1])
                # u = t + wvoc for this group, cast to bf16 for the PE
                sl = slice(ch * TCH, (ch + 1) * TCH)
                usl = scr.tile([128, TCH], f32, tag="usl")
                nc.vector.tensor_tensor(
                    out=usl[:], in0=t_raw[:, sl], in1=wvoc_sb[:, sl],
                    op=mybir.AluOpType.add)
                nc.vector.tensor_copy(out=u_bf[:, sl], in_=usl[:])
                for t in range(TCH):
                    k = ch * TCH + t
                    nc.tensor.matmul(
                        psA[:], u_bf[:, k:k + 1], pchunk[:, t, 0:B // 2],
                        start=(k == 0), stop=False)
                    nc.tensor.matmul(
                        psB[:], u_bf[:, k:k + 1], pchunk[:, t, B // 2:B],
                        start=(k == 0), stop=False)

            # --- dup correction: gather 256B t-blocks, extract, spread ---
            nc.sync.dma_start(
                t_dram[:].rearrange("q e -> (q e)").rearrange("(p k) -> p k", k=KC),
                t_raw[:])
            g = sb1.tile([128, 1, GBLK], f32)
            nc.gpsimd.dma_gather(
                g[:], t_dram[:], didx_sb[:],
                num_idxs=DUPN, num_idxs_reg=DUPN, elem_size=GBLK)
            dpo = scr.tile([128, 1, GBLK], f32, tag="dpo")
            dval = sb1.tile([128, 1], f32)
            nc.vector.scalar_tensor_tensor(
                out=dpo[:], in0=g[:], scalar=1.0, in1=dwv_sb[:],
                op0=mybir.AluOpType.mult, op1=mybir.AluOpType.mult,
                accum_out=dval[:])
            dval_bf = sb1.tile([128, 1], bf16)
            nc.vector.tensor_copy(out=dval_bf[:], in_=dval[:])
            nc.tensor.matmul(psA[:], dval_bf[:], dspread_sb[:, 0:B // 2],
                             start=False, stop=True)
            nc.tensor.matmul(psB[:], dval_bf[:], dspread_sb[:, B // 2:B],
                             start=False, stop=True)

            # --- partial logits -> DRAM -> ReduceScatter(add) ---
            acc_sb = sb1.tile([1, B], f32)
            nc.vector.tensor_copy(out=acc_sb[:, 0:B // 2], in_=psA[:])
            nc.vector.tensor_copy(out=acc_sb[:, B // 2:B], in_=psB[:])
            nc.sync.dma_start(accd[:], acc_sb[:])
            nc.gpsimd.collective_compute(
                "ReduceScatter",
                mybir.AluOpType.add,
                replica_groups=[list(range(N_CORES))],
                ins=[accd.opt()],
                outs=[rsd.opt()],
            )

            # --- sigmoid(logit + b) for this core's 128 rows ---
            rs_sb = sb1.tile([1, B // N_CORES], f32)
            nc.sync.dma_start(rs_sb[:], rsd[:])
            res = sb1.tile([1, B // N_CORES], f32)
            nc.scalar.activation(
                out=res[:], in_=rs_sb[:],
                func=mybir.ActivationFunctionType.Sigmoid,
                bias=b_sb[:], scale=1.0)
            nc.scalar.dma_start(outp.ap(), res[:])

    nc.compile()
    return nc


def kernel(x, emb_table, W, b):
    global _BUILT, LAST_RUN
    if _BUILT is None:
        _BUILT = _build()
    nc = _BUILT

    x = np.asarray(x)
    emb_table = np.ascontiguousarray(np.asarray(emb_table, dtype=np.float32))
    W = np.asarray(W, dtype=np.float32)
    b = np.asarray(b, dtype=np.float32)

    wemb = np.ascontiguousarray(W[:, :E])                  # [1, E]
    wv_full = W[0, E:]                                     # [V]
    bias_np = b.reshape(1, 1)

    # token -> (core, k, p, row) index decomposition
    rows_i = np.repeat(np.arange(B), S)
    v = x.reshape(-1).astype(np.int64)
    core = v // VPC
    vloc = v - core * VPC
    kk = vloc // 128
    pp = vloc - kk * 128

    # duplicate detection: count per (core, row, vloc)
    key = (core * B + rows_i) * VPC + vloc
    ukey, cnt = np.unique(key, return_counts=True)
    dup_sel = cnt >= 2
    d_key = ukey[dup_sel]
    d_extra = (cnt[dup_sel] - 1).astype(np.float32)
    d_core = d_key // (B * VPC)
    d_row = (d_key // VPC) % B
    d_vloc = d_key % VPC

    in_maps = []
    for c in range(N_CORES):
        tbl_np = np.zeros((VPAD, E), dtype=np.float32)
        tbl_np[:VPC] = emb_table[c * VPC:(c + 1) * VPC]
        wvs = np.zeros(VPAD, dtype=np.float32)
        wvs[:VPC] = wv_full[c * VPC:(c + 1) * VPC]
        wvoc_np = np.ascontiguousarray(wvs.reshape(KC, 128).T)  # [128, KC]

        m = core == c
        pres_np = np.zeros((KC, 128, B), dtype=ml_dtypes.float8_e4m3)
        pres_np[kk[m], pp[m], rows_i[m]] = 1.0

        dm = d_core == c
        nd = int(dm.sum())
        assert nd <= DUPN, f"core {c}: {nd} dup slots > {DUPN}"
        # t table flat position (p-major [128, KC]) -> 256B gather block
        dp = d_vloc[dm] % 128
        dk = d_vloc[dm] // 128
        flat = dp * KC + dk
        blk = (flat // GBLK).astype(np.int16)
        off = flat % GBLK

        bidx = np.zeros(DUPN, dtype=np.int16)
        bidx[:nd] = blk
        s_all = np.arange(DUPN)
        w16 = np.zeros((16, DUPN // 16), dtype=np.int16)
        w16[s_all % 16, s_all // 16] = bidx[s_all]
        didx_np = np.tile(w16, (8, 1))                      # [128, DUPN//16]

        dwv_np = np.zeros((128, 1, GBLK), dtype=ml_dtypes.bfloat16)
        dwv_np[np.arange(nd), 0, off] = d_extra[dm].astype(ml_dtypes.bfloat16)
        dspread_np = np.zeros((128, B), dtype=ml_dtypes.bfloat16)
        dspread_np[np.arange(nd), d_row[dm]] = 1.0

        in_maps.append({
            "tbl": tbl_np,
            "wemb": wemb,
            "wvoc": wvoc_np,
            "pres": pres_np,
            "didx": didx_np,
            "dwv": dwv_np,
            "dspread": dspread_np,
            "bias": bias_np,
        })

    LAST_RUN = run_bass_kernel_spmd(nc, in_maps, core_ids=list(range(N_CORES)))
    out = np.concatenate(
        [LAST_RUN.results[c]["outp"].reshape(B // N_CORES)
         for c in range(N_CORES)]
    )
    return out.reshape(B, 1)
